# revision 9
# baseline (speedup 1.0000x reference)
"""Trainium2 Bass kernel for nn_Criterion_24489903522258 (Circle-style loss).

Strategy (8 NeuronCores, data-parallel over rows of the similarity matrix):
  - Host builds A = [x_bf16, 32*onehot(labels)], B = [x_bf16, -32*onehot(labels)]
    so the PE computes u = A @ B^T = sim - 1024*same in one fused GEMM
    (label-equality mask folded into the contraction; one-hot in bf16 is exact).
  - By symmetry of sim/same, all per-COLUMN reductions of the reference become
    per-ROW reductions, so each core independently processes its 512 rows
    (4 tiles of 128 partitions x 4096).
  - Per row-tile: PE matmuls -> PSUM; copy to SBUF; DVE min/max reduces give
    pos_bound/neg_bound; ACT computes exp(40u-20) and exp(-2u-2047) (the -1024
    same-shift auto-zeroes the wrong class side of each); fused
    scalar_tensor_tensor applies the margin threshold masks and accumulates
    the per-row exp-sums.
  - The logsumexp max-shift cancels algebraically (vals = log(sum exp(w)), all
    w bounded), so no per-column ref is needed; host finishes the tiny
    O(BS) tail: nz gates, log, softplus, masked means.
"""

import os

import numpy as np
import ml_dtypes

import concourse.bass as bass
import concourse.bacc as bacc
import concourse.mybir as mybir
import concourse.tile as tile
from concourse.bass_utils import run_bass_kernel_spmd

BS, DIM, NCLS = 4096, 512, 100
NCORES = 8
RPC = BS // NCORES          # 512 rows per core
NT = RPC // 128             # 4 row-tiles per core
KPAD = 640                  # 512 + 100 padded to 5*128
KT = KPAD // 128
ALPHA = 32.0                # ALPHA^2 = 1024 = same-shift
SHIFT = np.float32(1024.0)
MARGIN = np.float32(0.1)

F32 = mybir.dt.float32
BF16 = mybir.dt.bfloat16
AF = mybir.ActivationFunctionType
ALU = mybir.AluOpType

# STT (masked accumulate) engine: "gpsimd" or "vector"
STT_ENGINE = os.environ.get("K_STT_ENGINE", "vector")
# which engine copies each PSUM half: list of 2 entries from {"scalar","vector"}
COPY_ENGINES = os.environ.get("K_COPY_ENGINES", "scalar,vector").split(",")

_built = None  # (nc,) cache


def _build_module():
    nc = bacc.Bacc()
    aT = nc.declare_dram_parameter("aT", [KPAD, RPC], BF16, isOutput=False)
    bT = nc.declare_dram_parameter("bT", [KPAD, BS], BF16, isOutput=False)
    out = nc.declare_dram_parameter("stats", [128, NT * 4], F32, isOutput=True)

    with tile.TileContext(nc) as tc:
        import contextlib
        with contextlib.ExitStack() as ctx:
            wp = ctx.enter_context(tc.tile_pool(name="weights", bufs=1))
            pp = ctx.enter_context(tc.tile_pool(name="psum", bufs=2, space="PSUM"))
            up = ctx.enter_context(tc.tile_pool(name="usb", bufs=2))
            ep = ctx.enter_context(tc.tile_pool(name="expo", bufs=3))
            scp = ctx.enter_context(tc.tile_pool(name="scratch", bufs=2))
            smp = ctx.enter_context(tc.tile_pool(name="small", bufs=8))
            stp = ctx.enter_context(tc.tile_pool(name="stats", bufs=2))

            cst = ctx.enter_context(tc.tile_pool(name="consts", bufs=1))
            bias_n = cst.tile([128, 1], F32, tag="bias_n")
            nc.vector.memset(bias_n, -20.0)
            bias_p = cst.tile([128, 1], F32, tag="bias_p")
            nc.vector.memset(bias_p, -2047.0)

            bts, ats = [], []
            for k in range(KT):
                tb = wp.tile([128, BS], BF16, tag=f"bt{k}")
                nc.sync.dma_start(out=tb, in_=bT[k * 128:(k + 1) * 128, :])
                bts.append(tb)
                ta = wp.tile([128, RPC], BF16, tag=f"at{k}")
                nc.sync.dma_start(out=ta, in_=aT[k * 128:(k + 1) * 128, :])
                ats.append(ta)

            for t in range(NT):
                usb = up.tile([128, BS], F32, tag="usb")
                for h in range(2):
                    ps = pp.tile([128, BS // 2], F32, tag="ps")
                    for k in range(KT):
                        for n in range(4):
                            nchunk = h * 4 + n
                            nc.tensor.matmul(
                                ps[:, n * 512:(n + 1) * 512],
                                lhsT=ats[k][:, t * 128:(t + 1) * 128],
                                rhs=bts[k][:, nchunk * 512:(nchunk + 1) * 512],
                                start=(k == 0),
                                stop=(k == KT - 1),
                            )
                    eng = nc.scalar if COPY_ENGINES[h] == "scalar" else nc.vector
                    if COPY_ENGINES[h] == "scalar":
                        eng.copy(out=usb[:, h * 2048:(h + 1) * 2048], in_=ps)
                    else:
                        eng.tensor_copy(out=usb[:, h * 2048:(h + 1) * 2048], in_=ps)

                ost = stp.tile([128, 4], F32, tag="ost")
                # bounds: pb_raw = min(u), nb = max(u)
                nc.vector.tensor_reduce(
                    out=ost[:, 0:1], in_=usb, axis=mybir.AxisListType.X, op=ALU.min)
                nc.vector.tensor_reduce(
                    out=ost[:, 1:2], in_=usb, axis=mybir.AxisListType.X, op=ALU.max)
                # thresholds
                thr_n = smp.tile([128, 1], F32, tag="thrn")
                nc.vector.tensor_scalar(
                    out=thr_n, in0=ost[:, 0:1], scalar1=1024.0, scalar2=0.1,
                    op0=ALU.add, op1=ALU.subtract)
                thr_p = smp.tile([128, 1], F32, tag="thrp")
                nc.vector.tensor_scalar(
                    out=thr_p, in0=ost[:, 1:2], scalar1=1024.0, scalar2=0.1,
                    op0=ALU.subtract, op1=ALU.add)

                # exp tensors (ACT): En = exp(40u - 20); Ep = exp(-2u - 2047)
                En = ep.tile([128, BS], F32, tag="E")
                nc.scalar.activation(out=En, in_=usb, func=AF.Exp,
                                     bias=bias_n, scale=40.0)
                Ep = ep.tile([128, BS], F32, tag="E")
                nc.scalar.activation(out=Ep, in_=usb, func=AF.Exp,
                                     bias=bias_p, scale=-2.0)

                stt_eng = nc.gpsimd if STT_ENGINE == "gpsimd" else nc.vector
                scr_n = scp.tile([128, BS], BF16, tag="scr")
                stt_eng.scalar_tensor_tensor(
                    out=scr_n, in0=usb, scalar=thr_n, in1=En,
                    op0=ALU.is_gt, op1=ALU.mult, accum_out=ost[:, 3:4])
                scr_p = scp.tile([128, BS], BF16, tag="scr")
                stt_eng.scalar_tensor_tensor(
                    out=scr_p, in0=usb, scalar=thr_p, in1=Ep,
                    op0=ALU.is_lt, op1=ALU.mult, accum_out=ost[:, 2:3])

                nc.sync.dma_start(out=out[:, t * 4:(t + 1) * 4], in_=ost)
    nc.compile()
    return nc


def _prepare_inputs(batch, labels):
    x = np.asarray(batch, np.float32)
    lab = np.asarray(labels).astype(np.int64)
    xb = x.astype(ml_dtypes.bfloat16)
    A = np.zeros((BS, KPAD), ml_dtypes.bfloat16)
    A[:, :DIM] = xb
    A[np.arange(BS), DIM + lab] = ml_dtypes.bfloat16(ALPHA)
    AT = np.ascontiguousarray(A.T)                      # (640, 4096)
    BT = AT.copy()
    BT[DIM:DIM + NCLS, :] = -BT[DIM:DIM + NCLS, :]      # negate one-hot rows
    in_maps = []
    for c in range(NCORES):
        in_maps.append({
            "aT": np.ascontiguousarray(AT[:, c * RPC:(c + 1) * RPC]),
            "bT": BT,
        })
    return in_maps


LAST_RESULTS = None  # test harness reads exec_time_ns from here


def kernel(batch, labels):
    global _built, LAST_RESULTS
    if _built is None:
        _built = _build_module()
    nc = _built
    in_maps = _prepare_inputs(batch, labels)
    res = run_bass_kernel_spmd(nc, in_maps, core_ids=list(range(NCORES)))
    LAST_RESULTS = res

    pb_raw = np.empty(BS, np.float32)
    nb = np.empty(BS, np.float32)
    s_pos = np.empty(BS, np.float32)
    s_neg = np.empty(BS, np.float32)
    for c in range(NCORES):
        st = res.results[c]["stats"]                    # [128, NT*4]
        for t in range(NT):
            rows = slice(c * RPC + t * 128, c * RPC + (t + 1) * 128)
            pb_raw[rows] = st[:, t * 4 + 0]
            nb[rows] = st[:, t * 4 + 1]
            s_pos[rows] = st[:, t * 4 + 2]
            s_neg[rows] = st[:, t * 4 + 3]

    # host tail (O(BS)): nz gates, vals=log(s), softplus, masked means
    pb = (pb_raw + SHIFT).astype(np.float32)
    nz_n = (nb + MARGIN) > pb
    nz_p = (pb - MARGIN) < nb
    vals_n = np.log(np.where(s_neg > 0, s_neg, 1.0).astype(np.float32))
    vals_p = np.log(np.where(s_pos > 0, s_pos, 1.0).astype(np.float32))

    def softplus(v):
        return np.logaddexp(0.0, v.astype(np.float64))

    def masked_mean(vals, nz, w):
        cnt = int(nz.sum())
        if cnt == 0:
            return float(np.logaddexp(0.0, 0.0)) / w
        return float(np.where(nz, softplus(vals) / w, 0.0).sum()) / cnt

    loss = masked_mean(vals_p, nz_p, 2.0) + masked_mean(vals_n, nz_n, 40.0)
    return np.float32(loss)
